# revision 12
# baseline (speedup 1.0000x reference)
"""Trainium2 Bass kernel for CubeFaceNN.

Computes, for x of shape [8, 1, 128, 128, 128] (f32):
    out[b, i, p] = relu(x[b, 0, p] - x[b, 0, p + OFF[i]])   (zero padded)
with OFF = [(0,-1,-1), (-1,0,-1), (1,-1,-1), (-1,1,-1), (-1,-1,0), (-1,-1,1)]
(derived from the reference's adj % 3 - 1 indexing).

Sharding: pure data parallel — batch b -> NeuronCore b (8 cores).

Layout: depth d on the 128 SBUF partitions, (h, w) in the free dims.
Pipeline per core:
  1. x (f32) -> SBUF via 4x 2 MiB HWDGE DMAs on the two rings (nc.sync /
     nc.scalar) — measured ~400 GB/s combined for ready-to-run loads,
     and it keeps the SWDGE path + GpSimd Q7 cores free for stores.
  2. ACT casts x -> xt16 (fp16). All further compute is fp16: the 2e-2
     error gate admits fp16 rounding of the inputs (max-norm error
     <= ~2^-10 * |x| ~ 7e-4 rel), and fp16 doubles DVE throughput and
     runs the PE shift at full rate.
  3. PE builds the partition-shifted copy xp16[d] = xt16[d+1] with a
     one-subdiagonal shift matrix (one-hot rows -> values exact);
     ACT copies PSUM -> xp16.
  4. 12 h-half units (6 channels x 2): DVE subs (flat contiguous APs),
     relu on DVE (fp16 2x) for ch0/2/4/5 and on ACT for ch1/3; boundary
     strips relu(A) patch the rows/cols where the shifted source is zero
     padding; the five od != 0 channels use the substituted frame
     out[i, d'+1] = relu(xp16[d'] - xt16[d', h+oh, w+ow]).
  5. Stores (fp16, 24.6 MiB total): SWDGE half-partition pairs with
     16 KiB descriptors (measured 22-24 GB/s/engine when queues stay
     deep); every 3rd unit goes to a HWDGE ring instead, cutting Q7
     descriptor-emission time (~25 ns/descriptor) and deepening queues.
  d-boundary planes out[i, 0] = relu(x[0]) (od=-1) and out[2, 127] =
  relu(x[127]) are handled separately in [h, w] layout on the rings.
"""

import numpy as np

import concourse.bacc as bacc
import concourse.mybir as mybir
import concourse.tile as tile
from concourse.bass_utils import run_bass_kernel_spmd

D = H = W = 128
HW = H * W
HALF = 64
UH = 64  # unit = h-half
UF = UH * W
NU = H // UH
N_CORES = 8
MMF = 512  # matmul moving free size (one PSUM bank of f32)
NCHUNK = HW // MMF
F32 = mybir.dt.float32
F16 = mybir.dt.float16

# (od, oh, ow) per output channel
OFFSETS = [(0, -1, -1), (-1, 0, -1), (1, -1, -1), (-1, 1, -1), (-1, -1, 0), (-1, -1, 1)]
DVE_RELU = (0, 2, 4, 5)

_NC_CACHE = {}


def build_nc(debug=False):
    nc = bacc.Bacc("TRN2", target_bir_lowering=False, debug=debug)
    x = nc.dram_tensor("x", [D, H, W], F32, kind="ExternalInput")
    out = nc.dram_tensor("out", [6, D, H, W], F16, kind="ExternalOutput")
    # shift matrix: sh[k, m] = 1 iff k == m+1, so (sh.T @ v)[m] = v[m+1]
    sh_dram = nc.inline_tensor(np.eye(D, k=-1, dtype=np.float16), name="shift")

    sub = mybir.AluOpType.subtract
    relu = mybir.ActivationFunctionType.Relu
    rings = [nc.sync, nc.scalar]

    with tile.TileContext(nc) as tc:
        with (
            tc.tile_pool(name="xt32", bufs=1) as xt32_pool,
            tc.tile_pool(name="xt16", bufs=1) as xt16_pool,
            tc.tile_pool(name="xp16", bufs=1) as xp16_pool,
            tc.tile_pool(name="sh", bufs=1) as sh_pool,
            tc.tile_pool(name="och", bufs=4) as och_pool,
            tc.tile_pool(name="pf32", bufs=2) as pf32_pool,
            tc.tile_pool(name="pf16", bufs=2) as pf16_pool,
            tc.tile_pool(name="ps", bufs=8, space="PSUM") as ps_pool,
        ):
            sht = sh_pool.tile([D, D], F16)
            nc.sync.dma_start(out=sht[:], in_=sh_dram[:])

            # x -> SBUF f32: 4x 2 MiB full-partition HWDGE DMAs (16 KiB
            # descriptors), alternating the two rings
            xt32 = xt32_pool.tile([D, H, W], F32)
            for c in range(4):
                hsl = slice(c * 32, (c + 1) * 32)
                rings[c % 2].dma_start(out=xt32[:, hsl], in_=x[:, hsl])
            xt32_2 = xt32.rearrange("d h w -> d (h w)")

            # fp16 working copy (ACT cast, chunked so PE can start early)
            xt16 = xt16_pool.tile([D, H, W], F16)
            xt2 = xt16.rearrange("d h w -> d (h w)")
            CAST = HW // 8
            for j in range(8):
                fsl = slice(j * CAST, (j + 1) * CAST)
                nc.scalar.copy(out=xt2[:, fsl], in_=xt32_2[:, fsl])

            # d-boundary planes: out[i, 0] = relu(x[0]) for od=-1 channels,
            # out[2, 127] = relu(x[127]); h on partitions, ring DMAs.
            p0s = pf32_pool.tile([H, W], F32)
            p0 = pf16_pool.tile([H, W], F16)
            nc.sync.dma_start(out=p0s[:], in_=x[0])
            nc.scalar.activation(p0[:], p0s[:], relu)
            p1s = pf32_pool.tile([H, W], F32)
            p1 = pf16_pool.tile([H, W], F16)
            nc.scalar.dma_start(out=p1s[:], in_=x[D - 1])
            nc.scalar.activation(p1[:], p1s[:], relu)
            for i, (od, _, _) in enumerate(OFFSETS):
                if od == -1:
                    rings[i % 2].dma_start(out=out[i, 0], in_=p0[:])
            nc.scalar.dma_start(out=out[2, D - 1], in_=p1[:])

            # xp16[d] = xt16[d+1] via PE shift matmul (fp16 one-hot, exact)
            xp16 = xp16_pool.tile([D, H, W], F16)
            xp2 = xp16.rearrange("d h w -> d (h w)")
            ps_tiles = []
            for c in range(NCHUNK):
                ps = ps_pool.tile([D, MMF], F32)
                nc.tensor.matmul(
                    out=ps[:],
                    lhsT=sht[:],
                    rhs=xt2[:, c * MMF : (c + 1) * MMF],
                    start=True,
                    stop=True,
                )
                ps_tiles.append(ps)

            copy_next = 0

            def emit_copies(n):
                nonlocal copy_next
                for c in range(copy_next, min(NCHUNK, copy_next + n)):
                    nc.scalar.copy(
                        out=xp2[:, c * MMF : (c + 1) * MMF], in_=ps_tiles[c][:]
                    )
                copy_next = min(NCHUNK, copy_next + n)

            def emit_compute(i, u, och):
                od, oh, ow = OFFSETS[i]
                dc = D if od == 0 else D - 1
                delta = oh * W + ow
                A3 = xp16 if od == -1 else xt16  # aligned with output frame
                S2 = xp2 if od == 1 else xt2  # d-shifted operand
                A2 = A3.rearrange("d h w -> d (h w)")

                hs, he = max(0, -oh), H - max(0, oh)
                f0, f1 = u * UF, (u + 1) * UF
                lo = max(f0, -delta)
                hi = min(f1, HW - delta)
                on_dve = i in DVE_RELU

                och2 = och.rearrange("d h w -> d (h w)")
                nc.vector.tensor_tensor(
                    out=och2[0:dc, lo - f0 : hi - f0],
                    in0=A2[0:dc, lo:hi],
                    in1=S2[0:dc, lo + delta : hi + delta],
                    op=sub,
                )

                # strips: shifted source is zero-padding there -> relu(A)
                def strip(osel, asel):
                    if on_dve:
                        nc.vector.tensor_scalar_max(och[osel], A3[asel], 0.0)
                    else:
                        nc.scalar.activation(och[osel], A3[asel], relu)

                r0 = u * UH
                if oh == -1 and u == 0:
                    strip((slice(0, dc), slice(0, 1)), (slice(0, dc), slice(0, 1)))
                if oh == 1 and u == NU - 1:
                    strip(
                        (slice(0, dc), slice(UH - 1, UH)),
                        (slice(0, dc), slice(H - 1, H)),
                    )
                if ow != 0:
                    wb = 0 if ow == -1 else W - 1
                    rs, re = max(hs, r0), min(he, r0 + UH)
                    strip(
                        (slice(0, dc), slice(rs - r0, re - r0), slice(wb, wb + 1)),
                        (slice(0, dc), slice(rs, re), slice(wb, wb + 1)),
                    )
                # interior relu (in place, fp16)
                osel = och2[0:dc, lo - f0 : hi - f0]
                if on_dve:
                    nc.vector.tensor_scalar_max(osel, osel, 0.0)
                else:
                    nc.scalar.activation(osel, osel, relu)

            unit_no = 0

            def emit_store(i, u, och):
                nonlocal unit_no
                od = OFFSETS[i][0]
                dc = D if od == 0 else D - 1
                d0 = 1 if od == -1 else 0
                r0 = u * UH
                dst = out[i, d0 : d0 + dc, r0 : r0 + UH]
                if unit_no % 3 == 2:
                    rings[unit_no % 2].dma_start(out=dst, in_=och[0:dc])
                else:
                    nc.gpsimd.dma_start(
                        out=out[i, d0 : d0 + HALF, r0 : r0 + UH], in_=och[0:HALF]
                    )
                    nc.gpsimd.dma_start(
                        out=out[i, d0 + HALF : d0 + dc, r0 : r0 + UH],
                        in_=och[HALF:dc],
                    )
                unit_no += 1

            # 2 waves of h-halves; xp copies interleaved into ACT's stream
            # so wave-u subs only queue behind the chunks they read
            SUB_ORDER = (0, 2, 1, 3, 4, 5)  # xp-free channel first
            STORE_ORDER = (0, 2, 4, 5, 1, 3)  # DVE-relu'd units first
            # ch0 needs no xp; all other wave-0 subs read xp chunks 0..16,
            # so those copies come right after ch0, the rest at wave tail
            PER_UNIT_COPIES = (17, 0, 0, 0, 7, 8)
            for u in range(NU):
                tiles = {}
                for j, i in enumerate(SUB_ORDER):
                    tiles[i] = och_pool.tile([D, UH, W], F16, name="och")
                    emit_compute(i, u, tiles[i])
                    emit_copies(PER_UNIT_COPIES[j])
                for i in STORE_ORDER:
                    emit_store(i, u, tiles[i])

    nc.compile()
    return nc


def _get_nc():
    if "nc" not in _NC_CACHE:
        _NC_CACHE["nc"] = build_nc()
    return _NC_CACHE["nc"]


def kernel(x: np.ndarray) -> np.ndarray:
    assert x.shape == (N_CORES, 1, D, H, W), x.shape
    nc = _get_nc()
    in_maps = [{"x": np.ascontiguousarray(x[b, 0], dtype=np.float32)} for b in range(N_CORES)]
    res = run_bass_kernel_spmd(nc, in_maps, core_ids=list(range(N_CORES)))
    return np.stack(
        [np.asarray(r["out"], dtype=np.float32) for r in res.results], axis=0
    )


# revision 13
# speedup vs baseline: 2.3223x; 2.3223x over previous
"""Trainium2 Bass kernel for CubeFaceNN.

Computes, for x of shape [8, 1, 128, 128, 128] (f32):
    out[b, i, p] = relu(x[b, 0, p] - x[b, 0, p + OFF[i]])   (zero padded)
with OFF = [(0,-1,-1), (-1,0,-1), (1,-1,-1), (-1,1,-1), (-1,-1,0), (-1,-1,1)]
(derived from the reference's adj % 3 - 1 indexing).

Sharding: pure data parallel — batch b -> NeuronCore b (8 cores).

Layout: depth d on the 128 SBUF partitions, (h, w) in the free dims.
Pipeline per core:
  1. x (f32) -> SBUF via 4x 2 MiB HWDGE DMAs on the two rings (nc.sync /
     nc.scalar) — measured ~400 GB/s combined for ready-to-run loads,
     and it keeps the SWDGE path + GpSimd Q7 cores free for stores.
  2. ACT casts x -> xt16 (fp16). All further compute is fp16: the 2e-2
     error gate admits fp16 rounding of the inputs (max-norm error
     <= ~2^-10 * |x| ~ 7e-4 rel), and fp16 doubles DVE throughput and
     runs the PE shift at full rate.
  3. PE builds the partition-shifted copy xp16[d] = xt16[d+1] with a
     one-subdiagonal shift matrix (one-hot rows -> values exact);
     ACT copies PSUM -> xp16.
  4. 12 h-half units (6 channels x 2): DVE subs (flat contiguous APs),
     relu on DVE (fp16 2x) for ch0/2/4/5 and on ACT for ch1/3; boundary
     strips relu(A) patch the rows/cols where the shifted source is zero
     padding; the five od != 0 channels use the substituted frame
     out[i, d'+1] = relu(xp16[d'] - xt16[d', h+oh, w+ow]).
  5. Stores (fp16, 24.6 MiB total): SWDGE half-partition pairs with
     16 KiB descriptors (measured 22-24 GB/s/engine when queues stay
     deep); every 3rd unit goes to a HWDGE ring instead, cutting Q7
     descriptor-emission time (~25 ns/descriptor) and deepening queues.
  d-boundary planes out[i, 0] = relu(x[0]) (od=-1) and out[2, 127] =
  relu(x[127]) are handled separately in [h, w] layout on the rings.
"""

import numpy as np

import concourse.bacc as bacc
import concourse.mybir as mybir
import concourse.tile as tile
from concourse.bass_utils import run_bass_kernel_spmd

D = H = W = 128
HW = H * W
HALF = 64
UH = 64  # unit = h-half
UF = UH * W
NU = H // UH
N_CORES = 8
MMF = 512  # matmul moving free size (one PSUM bank of f32)
NCHUNK = HW // MMF
F32 = mybir.dt.float32
F16 = mybir.dt.float16

# (od, oh, ow) per output channel
OFFSETS = [(0, -1, -1), (-1, 0, -1), (1, -1, -1), (-1, 1, -1), (-1, -1, 0), (-1, -1, 1)]
DVE_RELU = (0, 2, 4, 5)

_NC_CACHE = {}


def build_nc(debug=False):
    nc = bacc.Bacc("TRN2", target_bir_lowering=False, debug=debug)
    x = nc.dram_tensor("x", [D, H, W], F32, kind="ExternalInput")
    out = nc.dram_tensor("out", [6, D, H, W], F16, kind="ExternalOutput")
    # shift matrix: sh[k, m] = 1 iff k == m+1, so (sh.T @ v)[m] = v[m+1]
    sh_dram = nc.inline_tensor(np.eye(D, k=-1, dtype=np.float16), name="shift")

    sub = mybir.AluOpType.subtract
    relu = mybir.ActivationFunctionType.Relu
    rings = [nc.sync, nc.scalar]

    with tile.TileContext(nc) as tc:
        with (
            tc.tile_pool(name="xt32", bufs=1) as xt32_pool,
            tc.tile_pool(name="xt16", bufs=1) as xt16_pool,
            tc.tile_pool(name="xp16", bufs=1) as xp16_pool,
            tc.tile_pool(name="sh", bufs=1) as sh_pool,
            tc.tile_pool(name="och", bufs=4) as och_pool,
            tc.tile_pool(name="pf32", bufs=2) as pf32_pool,
            tc.tile_pool(name="pf16", bufs=2) as pf16_pool,
            tc.tile_pool(name="ps", bufs=8, space="PSUM") as ps_pool,
        ):
            sht = sh_pool.tile([D, D], F16)
            nc.sync.dma_start(out=sht[:], in_=sh_dram[:])

            # x -> SBUF f32: 4x 2 MiB full-partition HWDGE DMAs (16 KiB
            # descriptors), alternating the two rings
            xt32 = xt32_pool.tile([D, H, W], F32)
            for c in range(4):
                hsl = slice(c * 32, (c + 1) * 32)
                rings[c % 2].dma_start(out=xt32[:, hsl], in_=x[:, hsl])
            xt32_2 = xt32.rearrange("d h w -> d (h w)")

            # fp16 working copy (ACT cast, chunked so PE can start early)
            xt16 = xt16_pool.tile([D, H, W], F16)
            xt2 = xt16.rearrange("d h w -> d (h w)")
            CAST = HW // 8
            for j in range(8):
                fsl = slice(j * CAST, (j + 1) * CAST)
                nc.scalar.copy(out=xt2[:, fsl], in_=xt32_2[:, fsl])

            # d-boundary planes: out[i, 0] = relu(x[0]) for od=-1 channels,
            # out[2, 127] = relu(x[127]); h on partitions, ring DMAs.
            p0s = pf32_pool.tile([H, W], F32)
            p0 = pf16_pool.tile([H, W], F16)
            nc.sync.dma_start(out=p0s[:], in_=x[0])
            nc.scalar.activation(p0[:], p0s[:], relu)
            p1s = pf32_pool.tile([H, W], F32)
            p1 = pf16_pool.tile([H, W], F16)
            nc.scalar.dma_start(out=p1s[:], in_=x[D - 1])
            nc.scalar.activation(p1[:], p1s[:], relu)
            for i, (od, _, _) in enumerate(OFFSETS):
                if od == -1:
                    rings[i % 2].dma_start(out=out[i, 0], in_=p0[:])
            nc.scalar.dma_start(out=out[2, D - 1], in_=p1[:])

            # xp16[d] = xt16[d+1] via PE shift matmul (fp16 one-hot, exact)
            xp16 = xp16_pool.tile([D, H, W], F16)
            xp2 = xp16.rearrange("d h w -> d (h w)")
            ps_tiles = []
            for c in range(NCHUNK):
                ps = ps_pool.tile([D, MMF], F32)
                nc.tensor.matmul(
                    out=ps[:],
                    lhsT=sht[:],
                    rhs=xt2[:, c * MMF : (c + 1) * MMF],
                    start=True,
                    stop=True,
                )
                ps_tiles.append(ps)

            copy_next = 0

            def emit_copies(n):
                nonlocal copy_next
                for c in range(copy_next, min(NCHUNK, copy_next + n)):
                    nc.scalar.copy(
                        out=xp2[:, c * MMF : (c + 1) * MMF], in_=ps_tiles[c][:]
                    )
                copy_next = min(NCHUNK, copy_next + n)

            def emit_compute(i, u, och):
                od, oh, ow = OFFSETS[i]
                dc = D if od == 0 else D - 1
                delta = oh * W + ow
                A3 = xp16 if od == -1 else xt16  # aligned with output frame
                S2 = xp2 if od == 1 else xt2  # d-shifted operand
                A2 = A3.rearrange("d h w -> d (h w)")

                hs, he = max(0, -oh), H - max(0, oh)
                f0, f1 = u * UF, (u + 1) * UF
                lo = max(f0, -delta)
                hi = min(f1, HW - delta)
                on_dve = i in DVE_RELU

                och2 = och.rearrange("d h w -> d (h w)")
                nc.vector.tensor_tensor(
                    out=och2[0:dc, lo - f0 : hi - f0],
                    in0=A2[0:dc, lo:hi],
                    in1=S2[0:dc, lo + delta : hi + delta],
                    op=sub,
                )

                # strips: shifted source is zero-padding there -> relu(A)
                def strip(osel, asel):
                    if on_dve:
                        nc.vector.tensor_scalar_max(och[osel], A3[asel], 0.0)
                    else:
                        nc.scalar.activation(och[osel], A3[asel], relu)

                r0 = u * UH
                if oh == -1 and u == 0:
                    strip((slice(0, dc), slice(0, 1)), (slice(0, dc), slice(0, 1)))
                if oh == 1 and u == NU - 1:
                    strip(
                        (slice(0, dc), slice(UH - 1, UH)),
                        (slice(0, dc), slice(H - 1, H)),
                    )
                if ow != 0:
                    wb = 0 if ow == -1 else W - 1
                    rs, re = max(hs, r0), min(he, r0 + UH)
                    strip(
                        (slice(0, dc), slice(rs - r0, re - r0), slice(wb, wb + 1)),
                        (slice(0, dc), slice(rs, re), slice(wb, wb + 1)),
                    )
                # interior relu (in place, fp16)
                osel = och2[0:dc, lo - f0 : hi - f0]
                if on_dve:
                    nc.vector.tensor_scalar_max(osel, osel, 0.0)
                else:
                    nc.scalar.activation(osel, osel, relu)

            unit_no = 0

            def emit_store(i, u, och):
                nonlocal unit_no
                od = OFFSETS[i][0]
                dc = D if od == 0 else D - 1
                d0 = 1 if od == -1 else 0
                r0 = u * UH
                # only full-128-partition DMAs may ride the HWDGE rings: a
                # 127-partition ring DMA degenerates to single-engine
                # serial descriptor processing (~8x slower, trace-verified)
                if dc == D:
                    rings[unit_no % 2].dma_start(
                        out=out[i, d0 : d0 + dc, r0 : r0 + UH], in_=och[0:dc]
                    )
                else:
                    nc.gpsimd.dma_start(
                        out=out[i, d0 : d0 + HALF, r0 : r0 + UH], in_=och[0:HALF]
                    )
                    nc.gpsimd.dma_start(
                        out=out[i, d0 + HALF : d0 + dc, r0 : r0 + UH],
                        in_=och[HALF:dc],
                    )
                unit_no += 1

            # 2 waves of h-halves; xp copies interleaved into ACT's stream
            # so wave-u subs only queue behind the chunks they read
            SUB_ORDER = (0, 2, 1, 3, 4, 5)  # xp-free channel first
            STORE_ORDER = (0, 2, 4, 5, 1, 3)  # DVE-relu'd units first
            # ch0 needs no xp; all other wave-0 subs read xp chunks 0..16,
            # so those copies come right after ch0, the rest at wave tail
            PER_UNIT_COPIES = (17, 0, 0, 0, 7, 8)
            for u in range(NU):
                tiles = {}
                for j, i in enumerate(SUB_ORDER):
                    tiles[i] = och_pool.tile([D, UH, W], F16, name="och")
                    emit_compute(i, u, tiles[i])
                    emit_copies(PER_UNIT_COPIES[j])
                for i in STORE_ORDER:
                    emit_store(i, u, tiles[i])

    nc.compile()
    return nc


def _get_nc():
    if "nc" not in _NC_CACHE:
        _NC_CACHE["nc"] = build_nc()
    return _NC_CACHE["nc"]


def kernel(x: np.ndarray) -> np.ndarray:
    assert x.shape == (N_CORES, 1, D, H, W), x.shape
    nc = _get_nc()
    in_maps = [{"x": np.ascontiguousarray(x[b, 0], dtype=np.float32)} for b in range(N_CORES)]
    res = run_bass_kernel_spmd(nc, in_maps, core_ids=list(range(N_CORES)))
    return np.stack(
        [np.asarray(r["out"], dtype=np.float32) for r in res.results], axis=0
    )


# revision 17
# speedup vs baseline: 3.1265x; 1.3463x over previous
"""Trainium2 Bass kernel for CubeFaceNN.

Computes, for x of shape [8, 1, 128, 128, 128] (f32):
    out[b, i, p] = relu(x[b, 0, p] - x[b, 0, p + OFF[i]])   (zero padded)
with OFF = [(0,-1,-1), (-1,0,-1), (1,-1,-1), (-1,1,-1), (-1,-1,0), (-1,-1,1)]
(derived from the reference's adj % 3 - 1 indexing).

Sharding: pure data parallel — batch b -> NeuronCore b (8 cores).

Layout: depth d on the 128 SBUF partitions, (h, w) in the free dims.
Pipeline per core:
  1. x (f32) -> SBUF via 4x 2 MiB HWDGE DMAs on the two rings (nc.sync /
     nc.scalar) — ~400 GB/s combined for ready-to-run full-partition
     loads, keeping SWDGE + the GpSimd Q7 cores free for stores.
  2. ACT casts x -> xt16 (fp16, chunked). All compute is fp16: the 2e-2
     error gate admits fp16 input rounding (max-norm rel err ~6e-4) and
     fp16 runs DVE tensor ops ~1.5-2x faster and the PE shift at column
     rate.
  3. PE builds the partition-shifted copy xp16[d] = xt16[d+1] with a
     one-subdiagonal shift matrix (one-hot rows -> values exact);
     ACT copies PSUM -> xp16, interleaved into the unit waves.
  4. 24 h-quarter units (6 channels x 4 waves, 8 och buffers): DVE subs
     on flat contiguous APs; relu on DVE (ch0/2) or ACT (ch1/3/4/5);
     boundary strips relu(A) patch rows/cols where the shifted source is
     zero padding; od != 0 channels use the substituted frame
     out[i, d'+1] = relu(xp16[d'] - xt16[d', h+oh, w+ow]).
  5. Stores (fp16): ch0 and ch2 units are full-128-partition (ch2's
     missing d=127 output plane relu(x[127]) is computed INTO partition
     127 of its och tile) and ride the HWDGE rings; od=-1 channels
     (127 partitions) go SWDGE as half-partition pairs with 8 KiB
     descriptors — a 127-partition ring DMA degenerates to serial
     single-engine descriptor processing (trace-verified ~8x slower).
  The four od=-1 d=0 planes out[i, 0] = relu(x[0]) are stored from one
  [h, w]-layout tile on the rings.
"""

import numpy as np

import concourse.bacc as bacc
import concourse.mybir as mybir
import concourse.tile as tile
from concourse.bass_utils import run_bass_kernel_spmd

D = H = W = 128
HW = H * W
HALF = 64
UH = 32  # unit = h-quarter
UF = UH * W
NU = H // UH
N_CORES = 8
MMF = 512  # matmul moving free size (one PSUM bank of f32)
NCHUNK = HW // MMF
F32 = mybir.dt.float32
F16 = mybir.dt.float16

# (od, oh, ow) per output channel
OFFSETS = [(0, -1, -1), (-1, 0, -1), (1, -1, -1), (-1, 1, -1), (-1, -1, 0), (-1, -1, 1)]
DVE_RELU = (0, 2)

_NC_CACHE = {}


def build_nc(debug=False):
    nc = bacc.Bacc("TRN2", target_bir_lowering=False, debug=debug)
    x = nc.dram_tensor("x", [D, H, W], F32, kind="ExternalInput")
    out = nc.dram_tensor("out", [6, D, H, W], F16, kind="ExternalOutput")
    # shift matrix: sh[k, m] = 1 iff k == m+1, so (sh.T @ v)[m] = v[m+1]
    sh_dram = nc.inline_tensor(np.eye(D, k=-1, dtype=np.float16), name="shift")

    sub = mybir.AluOpType.subtract
    relu = mybir.ActivationFunctionType.Relu
    rings = [nc.sync, nc.scalar]

    with tile.TileContext(nc) as tc:
        with (
            tc.tile_pool(name="xt32", bufs=1) as xt32_pool,
            tc.tile_pool(name="xt16", bufs=1) as xt16_pool,
            tc.tile_pool(name="xp16", bufs=1) as xp16_pool,
            tc.tile_pool(name="sh", bufs=1) as sh_pool,
            tc.tile_pool(name="och", bufs=8) as och_pool,
            tc.tile_pool(name="pf32", bufs=1) as pf32_pool,
            tc.tile_pool(name="pf16", bufs=1) as pf16_pool,
            tc.tile_pool(name="ps", bufs=8, space="PSUM") as ps_pool,
        ):
            sht = sh_pool.tile([D, D], F16)
            nc.sync.dma_start(out=sht[:], in_=sh_dram[:])

            # x -> SBUF f32: 4x 2 MiB full-partition HWDGE DMAs (16 KiB
            # descriptors), alternating the two rings
            xt32 = xt32_pool.tile([D, H, W], F32)
            for c in range(4):
                hsl = slice(c * 32, (c + 1) * 32)
                rings[c % 2].dma_start(out=xt32[:, hsl], in_=x[:, hsl])
            xt32_2 = xt32.rearrange("d h w -> d (h w)")

            # fp16 working copy (ACT cast, chunked so PE + subs start early)
            xt16 = xt16_pool.tile([D, H, W], F16)
            xt2 = xt16.rearrange("d h w -> d (h w)")
            CAST = HW // 8
            for j in range(8):
                fsl = slice(j * CAST, (j + 1) * CAST)
                nc.scalar.copy(out=xt2[:, fsl], in_=xt32_2[:, fsl])

            # d=0 boundary planes for the od=-1 channels: out[i, 0] =
            # relu(x[0]); h on partitions, ring DMAs.
            p0s = pf32_pool.tile([H, W], F32)
            p0 = pf16_pool.tile([H, W], F16)
            nc.sync.dma_start(out=p0s[:], in_=x[0])
            nc.scalar.activation(p0[:], p0s[:], relu)
            for i, (od, _, _) in enumerate(OFFSETS):
                if od == -1:
                    rings[i % 2].dma_start(out=out[i, 0], in_=p0[:])

            # xp16[d] = xt16[d+1] via PE shift matmul (fp16 one-hot, exact)
            xp16 = xp16_pool.tile([D, H, W], F16)
            xp2 = xp16.rearrange("d h w -> d (h w)")
            ps_tiles = []
            for c in range(NCHUNK):
                ps = ps_pool.tile([D, MMF], F32)
                nc.tensor.matmul(
                    out=ps[:],
                    lhsT=sht[:],
                    rhs=xt2[:, c * MMF : (c + 1) * MMF],
                    start=True,
                    stop=True,
                )
                ps_tiles.append(ps)

            copy_next = 0

            def emit_copies(n):
                nonlocal copy_next
                for c in range(copy_next, min(NCHUNK, copy_next + n)):
                    nc.scalar.copy(
                        out=xp2[:, c * MMF : (c + 1) * MMF], in_=ps_tiles[c][:]
                    )
                copy_next = min(NCHUNK, copy_next + n)

            def emit_compute(i, u, och):
                od, oh, ow = OFFSETS[i]
                dc = D if od == 0 else D - 1
                delta = oh * W + ow
                A3 = xp16 if od == -1 else xt16  # aligned with output frame
                S2 = xp2 if od == 1 else xt2  # d-shifted operand
                A2 = A3.rearrange("d h w -> d (h w)")

                hs, he = max(0, -oh), H - max(0, oh)
                f0, f1 = u * UF, (u + 1) * UF
                lo = max(f0, -delta)
                hi = min(f1, HW - delta)
                on_dve = i in DVE_RELU

                och2 = och.rearrange("d h w -> d (h w)")
                r0 = u * UH
                if i == 2:
                    # ch2's d=127 output plane relu(x[127]) lands on och
                    # partition 127 so the store is full-128-partition.
                    # Compute bases must be 32-aligned, so relu a whole
                    # [96:128] block FIRST; the sub below then overwrites
                    # partitions 96..126 (same engine -> ordered).
                    nc.vector.tensor_scalar_max(
                        och[96:D, :, :], xt16[96:D, r0 : r0 + UH, :], 0.0
                    )
                nc.vector.tensor_tensor(
                    out=och2[0:dc, lo - f0 : hi - f0],
                    in0=A2[0:dc, lo:hi],
                    in1=S2[0:dc, lo + delta : hi + delta],
                    op=sub,
                )

                # strips: shifted source is zero-padding there -> relu(A)
                def strip(osel, asel):
                    if on_dve:
                        nc.vector.tensor_scalar_max(och[osel], A3[asel], 0.0)
                    else:
                        nc.scalar.activation(och[osel], A3[asel], relu)

                if oh == -1 and u == 0:
                    strip((slice(0, dc), slice(0, 1)), (slice(0, dc), slice(0, 1)))
                if oh == 1 and u == NU - 1:
                    strip(
                        (slice(0, dc), slice(UH - 1, UH)),
                        (slice(0, dc), slice(H - 1, H)),
                    )
                if ow != 0:
                    wb = 0 if ow == -1 else W - 1
                    rs, re = max(hs, r0), min(he, r0 + UH)
                    strip(
                        (slice(0, dc), slice(rs - r0, re - r0), slice(wb, wb + 1)),
                        (slice(0, dc), slice(rs, re), slice(wb, wb + 1)),
                    )
                # interior relu (in place, fp16)
                osel = och2[0:dc, lo - f0 : hi - f0]
                if on_dve:
                    nc.vector.tensor_scalar_max(osel, osel, 0.0)
                else:
                    nc.scalar.activation(osel, osel, relu)

            unit_no = 0

            def emit_store(i, u, och):
                nonlocal unit_no
                od = OFFSETS[i][0]
                r0 = u * UH
                if od == 0:  # ch0: full 128 partitions -> ring
                    rings[unit_no % 2].dma_start(
                        out=out[i, :, r0 : r0 + UH], in_=och[:]
                    )
                elif i == 2:  # ch2 + its d=127 plane: 128 partitions -> ring
                    rings[unit_no % 2].dma_start(
                        out=out[i, :, r0 : r0 + UH], in_=och[:]
                    )
                else:  # od=-1: 127 partitions -> SWDGE half pairs
                    nc.gpsimd.dma_start(
                        out=out[i, 1 : 1 + HALF, r0 : r0 + UH], in_=och[0:HALF]
                    )
                    nc.gpsimd.dma_start(
                        out=out[i, 1 + HALF : D, r0 : r0 + UH],
                        in_=och[HALF : D - 1],
                    )
                unit_no += 1

            # 4 waves of h-quarters; xp copies interleaved into ACT's
            # stream: the 9 chunks wave 0 reads come right after ch0's
            # unit, later waves' chunks trickle at each wave tail
            SUB_ORDER = (0, 2, 1, 3, 4, 5)  # xp-free channel first
            STORE_ORDER = (0, 2, 4, 5, 1, 3)  # ring + DVE-relu'd units first
            for u in range(NU):
                tiles = {}
                for j, i in enumerate(SUB_ORDER):
                    tiles[i] = och_pool.tile([D, UH, W], F16, name="och")
                    emit_compute(i, u, tiles[i])
                    if j == 0:
                        emit_copies(9 if u == 0 else 4)
                    elif j >= 4:
                        emit_copies(2)
                for i in STORE_ORDER:
                    emit_store(i, u, tiles[i])

    nc.compile()
    return nc


def _get_nc():
    if "nc" not in _NC_CACHE:
        _NC_CACHE["nc"] = build_nc()
    return _NC_CACHE["nc"]


def kernel(x: np.ndarray) -> np.ndarray:
    assert x.shape == (N_CORES, 1, D, H, W), x.shape
    nc = _get_nc()
    in_maps = [{"x": np.ascontiguousarray(x[b, 0], dtype=np.float32)} for b in range(N_CORES)]
    res = run_bass_kernel_spmd(nc, in_maps, core_ids=list(range(N_CORES)))
    return np.stack(
        [np.asarray(r["out"], dtype=np.float32) for r in res.results], axis=0
    )
